# revision 1
# baseline (speedup 1.0000x reference)
"""Contrastive loss kernel for Trainium2 (8 NeuronCores, SPMD).

Math: loss = mean_{pos pairs}(1-cos_sim)^2 + mean_{neg pairs}relu(cos_sim-1)^2
with pos = same-label upper-triangle pairs, neg = different-label ordered pairs.

Strategy:
  * Host sorts rows by label so same-label pairs form a narrow diagonal band,
    and rotates columns per core so the band lands at the same local columns
    on every core (one uniform SPMD program).
  * Each core computes its [512, 4096] slice of the Gram matrix in bf16 on
    the PE (raw, unnormalized rhs; lhsT pre-scaled by 1/norm).
  * Norms come from a row-major squared-sum pipeline (ScalarE activation
    accumulate), inverted on VectorE in a compact [128, 32] layout, and
    broadcast along partitions via a K=1 ones-matmul.
  * Epilogue per PSUM tile: multiply by inv_j (column side of normalization),
    relu(s-1) then Square-accumulate => neg partials over ALL pairs; on the
    diagonal band only, index masks (computed from targets-derived per-row
    bounds) give the pos partials and a same-label correction to subtract
    from the neg sum.
  * Exact pair counts are integer combinatorics of targets, done on host.
    Host combines 8 x [128, 16] partial-stat tensors into the final scalar.
"""

import numpy as np
import ml_dtypes

import concourse.bass as bass
import concourse.bacc as bacc
import concourse.mybir as mybir
import concourse.tile as tile

N, D, NCORES = 4096, 512, 8
RPC = N // NCORES  # 512 rows per core
BAND_W = 512       # band slice width (covers all same-label cols per strip)
BMAX = 192         # max same-label block size the fixed band supports

F32 = mybir.dt.float32
BF16 = mybir.dt.bfloat16
AF = mybir.ActivationFunctionType
ALU = mybir.AluOpType


def build_program():
    nc = bacc.Bacc(None)
    xt16_d = nc.declare_dram_parameter("xt16", [D, N], BF16, isOutput=False)
    xr16_d = nc.declare_dram_parameter("xr16", [D, N], BF16, isOutput=False)
    meta_d = nc.declare_dram_parameter("meta", [128, BAND_W + 16], F32,
                                       isOutput=False)
    stats_d = nc.declare_dram_parameter("stats", [128, 16], F32, isOutput=True)
    scratch = nc.dram_tensor("invbounce", [N], F32)

    with tile.TileContext(nc) as tc:
        with (
            tc.tile_pool(name="perm", bufs=1) as perm,
            tc.tile_pool(name="rows", bufs=4) as rows,
            tc.tile_pool(name="rjunk", bufs=2) as rjunk,
            tc.tile_pool(name="work", bufs=2) as work,
            tc.tile_pool(name="bandp", bufs=2) as bandp,
            tc.tile_pool(name="psum", bufs=2, space="PSUM") as psum,
        ):
            meta_t = perm.tile([128, BAND_W + 16], F32, tag="meta")
            nc.sync.dma_start(meta_t[:], meta_d[:])
            iota_t = meta_t[:, 0:BAND_W]
            aux_t = meta_t[:, BAND_W:BAND_W + 16]
            stats_t = perm.tile([128, 16], F32, tag="stats")
            sumsq = perm.tile([128, 32], F32, tag="sumsq")
            xt_c = [perm.tile([128, N], BF16, tag=f"xt{k}", name=f"xt{k}") for k in range(4)]
            invf = perm.tile([128, N], F32, tag="invf")
            inv16own = perm.tile([128, RPC], BF16, tag="inv16own")
            xtL = [perm.tile([128, RPC], BF16, tag=f"xtL{k}", name=f"xtL{k}") for k in range(4)]
            flatF = perm.tile([1, N], F32, tag="flatF")
            nrm = perm.tile([128, 32], F32, tag="nrm")
            nrmx = perm.tile([128, 32], F32, tag="nrmx")
            invr = perm.tile([128, 32], F32, tag="invr")

            # --- row-major norms pipeline (overlaps DMA) ---
            # slab g holds 8 row-tiles: partition p, cols [512t', 512t'+512)
            # = local column j = 32p + 8g + t'
            for g in range(4):
                rt = rows.tile([128, N], BF16, tag="rt", name=f"rt{g}")
                for hh in range(2):
                    nc.sync.dma_start(
                        rt[:, 2048 * hh:2048 * (hh + 1)],
                        xr16_d[128 * g:128 * (g + 1),
                               2048 * hh:2048 * (hh + 1)])
                for tp in range(8):
                    t = 8 * g + tp
                    jk = rjunk.tile([128, D], BF16, tag="rj", name=f"rj{t}")
                    sl = rt[:, 512 * tp:512 * (tp + 1)]
                    nc.scalar.activation(jk[:], sl, AF.Square,
                                          accum_out=sumsq[:, t:t + 1])

            # --- transposed-chunk DMAs, first halves first ---
            for h in range(2):
                for k in range(4):
                    nc.sync.dma_start(
                        xt_c[k][:, 2048 * h:2048 * (h + 1)],
                        xt16_d[128 * k:128 * (k + 1), 2048 * h:2048 * (h + 1)])

            # --- inv = 1/max(sqrt(sumsq), eps), in compact layout ---
            nc.scalar.activation(nrm[:], sumsq[:], AF.Sqrt)
            nc.vector.tensor_scalar(out=nrmx[:], in0=nrm[:], scalar1=1e-8,
                                    scalar2=None, op0=ALU.max)
            nc.vector.reciprocal(invr[:], nrmx[:])

            # --- reorder [128,32] -> [1,4096] (partition-gather DMA) ---
            nc.sync.dma_start(
                flatF[0:1, :].rearrange("o (p t) -> o p t", p=128), invr[:])

            # --- broadcast inv along partitions via K=1 f32r matmul ---
            onesK = perm.tile([1, 128], F32, tag="onesK")
            nc.vector.memset(onesK[:], 1.0)

            def bcast_half(h):
                bmg = psum.tile([128, 2048], F32, tag="mega", name=f"bmg{h}")
                for t4 in range(4):
                    nc.tensor.matmul(
                        bmg[:, 512 * t4:512 * (t4 + 1)],
                        onesK[0:1, :].bitcast(mybir.dt.float32r),
                        flatF[0:1, 2048 * h + 512 * t4:
                              2048 * h + 512 * (t4 + 1)].bitcast(
                                  mybir.dt.float32r),
                        start=True, stop=True)
                nc.scalar.activation(invf[:, 2048 * h:2048 * (h + 1)], bmg[:],
                                     AF.Copy)

            # h=0 first: inv16own lives in [256,768) so xtL can start while
            # the h=1 broadcast/copy overlaps with early Gram work
            bcast_half(0)
            nc.scalar.activation(inv16own[:], invf[:, 256:768], AF.Copy)
            for k in range(4):
                eng = nc.vector if k % 2 == 0 else nc.gpsimd
                eng.tensor_tensor(xtL[k][:], xt_c[k][:, 256:768],
                                  inv16own[:], ALU.mult)
            bcast_half(1)

            # --- Gram megatiles + epilogue ---
            for h in range(2):
                for s in range(4):
                    mi = 4 * h + s
                    sim = psum.tile([128, 2048], F32, tag="mega")
                    for k in range(4):
                        for t4 in range(4):
                            nc.tensor.matmul(
                                sim[:, 512 * t4:512 * (t4 + 1)],
                                xtL[k][:, 128 * s:128 * (s + 1)],
                                xt_c[k][:, 2048 * h + 512 * t4:
                                          2048 * h + 512 * (t4 + 1)],
                                start=(k == 0), stop=(k == 3))
                    sb = work.tile([128, 2048], BF16, tag="sb")
                    nc.vector.tensor_tensor(sb[:], sim[:],
                                            invf[:, 2048 * h:2048 * (h + 1)],
                                            ALU.mult)
                    rb = work.tile([128, 2048], BF16, tag="rb")
                    nc.vector.tensor_scalar(out=rb[:], in0=sb[:], scalar1=1.0,
                                            scalar2=0.0, op0=ALU.subtract,
                                            op1=ALU.max)
                    jk2 = work.tile([128, 2048], BF16, tag="jk")
                    nc.scalar.activation(jk2[:], rb[:], AF.Square,
                                         accum_out=stats_t[:, mi:mi + 1])
                    if h == 0:
                        a = 64 + 128 * s
                        u1 = bandp.tile([128, BAND_W], BF16, tag="u1")
                        nc.vector.tensor_scalar(out=u1[:], in0=sb[:, a:a + BAND_W],
                                                scalar1=1.0, scalar2=None,
                                                op0=ALU.subtract)
                        chi = bandp.tile([128, BAND_W], BF16, tag="chi")
                        nc.vector.tensor_scalar(out=chi[:], in0=iota_t,
                                                scalar1=aux_t[:, 4 * s + 2:4 * s + 3],
                                                scalar2=None, op0=ALU.is_lt)
                        b1 = bandp.tile([128, BAND_W], BF16, tag="b1")
                        nc.vector.tensor_scalar(out=b1[:], in0=iota_t,
                                                scalar1=aux_t[:, 4 * s:4 * s + 1],
                                                scalar2=None, op0=ALU.is_gt)
                        a1 = bandp.tile([128, BAND_W], BF16, tag="a1")
                        nc.vector.tensor_scalar(out=a1[:], in0=iota_t,
                                                scalar1=aux_t[:, 4 * s + 1:4 * s + 2],
                                                scalar2=None, op0=ALU.is_ge)
                        pu = bandp.tile([128, BAND_W], BF16, tag="pu")
                        nc.gpsimd.tensor_tensor(pu[:], b1[:], chi[:], ALU.mult)
                        tm = bandp.tile([128, BAND_W], BF16, tag="tm")
                        nc.gpsimd.tensor_tensor(tm[:], a1[:], chi[:], ALU.mult)
                        v = bandp.tile([128, BAND_W], BF16, tag="v")
                        nc.gpsimd.tensor_tensor(v[:], u1[:], pu[:], ALU.mult)
                        g = bandp.tile([128, BAND_W], BF16, tag="g")
                        nc.gpsimd.tensor_tensor(g[:], rb[:, a:a + BAND_W],
                                                tm[:], ALU.mult)
                        bj1 = bandp.tile([128, BAND_W], BF16, tag="bj1")
                        nc.scalar.activation(bj1[:], v[:], AF.Square,
                                             accum_out=stats_t[:, 8 + s:9 + s])
                        bj2 = bandp.tile([128, BAND_W], BF16, tag="bj2")
                        nc.scalar.activation(bj2[:], g[:], AF.Square,
                                             accum_out=stats_t[:, 12 + s:13 + s])

            nc.sync.dma_start(stats_d[:], stats_t[:])
    nc.finalize()
    return nc


def host_prepare(inputs, targets):
    """Sort/rotate/pack per-core inputs. Returns (in_maps, counts)."""
    inputs = np.asarray(inputs, np.float32)
    targets_np = np.asarray(targets)
    order = np.argsort(targets_np, kind="stable")
    tss = targets_np[order]
    X = inputs[order]
    lo = np.searchsorted(tss, tss, side="left").astype(np.int64)
    hi = np.searchsorted(tss, tss, side="right").astype(np.int64)
    bmax = int((hi - lo).max())
    if bmax > BMAX:
        raise NotImplementedError(
            f"label block of size {bmax} exceeds supported band ({BMAX})")

    X16 = X.astype(ml_dtypes.bfloat16)
    # slab layout [512, 4096]: slab g partition p cols [512t',512t'+512) hold
    # local column j = 32p + 8g + t', so sumsq[p, 8g+t'] = sumsq_j with
    # j = 32p + (8g+t') and the [128,32] inv tile flattens linearly through
    # the DRAM bounce
    g_idx = np.arange(4)[:, None, None]          # slab
    p_idx = np.arange(128)[None, :, None]        # partition
    tp_idx = np.arange(8)[None, None, :]         # tile-in-slab
    j_map = (32 * p_idx + 8 * g_idx + tp_idx)    # [4, 128, 8]


    in_maps = []
    for c in range(NCORES):
        off = (RPC * c - 256) % N
        colmap = (np.arange(N) + off) % N  # local j -> global sorted row
        Xc = X16[colmap, :]
        xt16_c = np.ascontiguousarray(Xc.T)
        # [4, 128, 8, 512] -> [512, 4096]
        xr16_c = np.ascontiguousarray(
            Xc[j_map, :].reshape(4, 128, 8 * D).reshape(512, 4096))
        meta = np.zeros((128, BAND_W + 16), np.float32)
        meta[:, 0:BAND_W] = np.arange(BAND_W, dtype=np.float32)[None, :]
        aux = meta[:, BAND_W:BAND_W + 16]
        for s in range(4):
            a_s = 64 + 128 * s
            gi = RPC * c + 128 * s + np.arange(128)
            base = RPC * c - 256 + a_s
            i_cmp = (gi - base).astype(np.float32)
            lo_cmp = (lo[gi] - base).astype(np.float32)
            hi_cmp = (hi[gi] - base).astype(np.float32)
            assert (lo_cmp >= 0).all() and (hi_cmp <= BAND_W).all()
            aux[:, 4 * s + 0] = i_cmp
            aux[:, 4 * s + 1] = lo_cmp
            aux[:, 4 * s + 2] = hi_cmp
        in_maps.append({
            "xt16": xt16_c,
            "xr16": xr16_c,
            "meta": meta,
        })

    cnts = np.bincount(targets_np.astype(np.int64))
    pos_cnt = float((cnts * (cnts - 1) // 2).sum())
    neg_cnt = float(N * N - (cnts * cnts).sum())
    return in_maps, pos_cnt, neg_cnt


def combine(stats_list, pos_cnt, neg_cnt):
    neg_all = 0.0
    pos_sum = 0.0
    corr = 0.0
    for st in stats_list:
        st = np.asarray(st, np.float64)
        neg_all += st[:, 0:8].sum()
        pos_sum += st[:, 8:12].sum()
        corr += st[:, 12:16].sum()
    loss = np.float32(pos_sum / pos_cnt + (neg_all - corr) / neg_cnt)
    return np.asarray(loss, np.float32)


_prog_cache = {}


def kernel(inputs, targets):
    from concourse.bass_utils import run_bass_kernel_spmd
    in_maps, pos_cnt, neg_cnt = host_prepare(inputs, targets)
    if "nc" not in _prog_cache:
        _prog_cache["nc"] = build_program()
    nc = _prog_cache["nc"]
    res = run_bass_kernel_spmd(nc, in_maps, list(range(NCORES)))
    stats_list = [res.results[c]["stats"] for c in range(NCORES)]
    return combine(stats_list, pos_cnt, neg_cnt)



# revision 9
# speedup vs baseline: 2.8477x; 2.8477x over previous
"""Contrastive loss kernel for Trainium2 (8 NeuronCores, SPMD).

Math: loss = mean_{pos pairs}(1-cos_sim)^2 + mean_{neg pairs}relu(cos_sim-1)^2
with pos = same-label upper-triangle pairs, neg = different-label ordered pairs.

Since cos_sim(x_i, x_j) <= 1 for all pairs (Cauchy-Schwarz on normalized
vectors, with strict inequality for non-parallel vectors), relu(cos_sim-1) is
identically zero on every neg pair, so the neg term contributes exactly 0/neg_cnt.
Only the pos term needs computing, and pos pairs are confined to same-label
blocks.

Strategy:
  * Host sorts rows by label (stable), so same-label pairs form contiguous
    blocks along the diagonal. Each core owns 512 rows and computes, for each
    of its rows i, the Gram entries for columns j in (i, hi_i) -- a 224-wide
    band per 128-row strip (supports label blocks up to 97 rows; actual max
    is ~82 for this distribution).
  * Per core DMA: a [512, 640] fp16 window of X^T (own rows + 128 overflow
    columns into the next core, wrapped mod 4096).
  * Norms: square the window chunks (vector), reduce over D with 20 small
    [128,128]x[128,1] ones-matmuls into a column-major [128, 5] PSUM tile,
    then sqrt / clamp / reciprocal. A PE transpose + five K=1 ones-matmuls
    broadcast the inverse norms along partitions for the column side.
  * Gram: raw fp16 band matmuls (4 strips x 4 K-chunks, [128x128]x[128x224]).
  * Epilogue per strip: s = G * inv_j (vector) * inv_i - 1 (scalar activation
    with per-partition scale), mask j < hi_i (iota/tensor_scalar) and j > i
    (affine_select), then a fused square-and-accumulate (tensor_tensor_reduce)
    into per-partition partial sums.
  * Host sums the 8 x [128, 4] partials and divides by the exact pair count
    (integer combinatorics of targets).
"""

import numpy as np

import concourse.bass as bass
import concourse.bacc as bacc
import concourse.mybir as mybir
import concourse.tile as tile

N, D, NCORES = 4096, 512, 8
RPC = N // NCORES   # 512 rows per core
W = 640             # window columns per core
BW = 224            # band width per 128-row strip
NSTRIP = RPC // 128
NCHUNK = D // 128

F32 = mybir.dt.float32
F16 = mybir.dt.float16
AF = mybir.ActivationFunctionType
ALU = mybir.AluOpType


def build_program():
    nc = bacc.Bacc(None)
    xt_d = nc.declare_dram_parameter("xt", [D, W], F16, isOutput=False)
    aux_d = nc.declare_dram_parameter("aux", [128, 8], F32, isOutput=False)
    stats_d = nc.declare_dram_parameter("stats", [128, 4], F32, isOutput=True)

    with tile.TileContext(nc) as tc:
        with (
            tc.tile_pool(name="const", bufs=1) as const,
            tc.tile_pool(name="work", bufs=2) as work,
            tc.tile_pool(name="psA", bufs=1, space="PSUM") as psA,
            tc.tile_pool(name="psG", bufs=3, space="PSUM") as psG,
        ):
            # --- tiny constants / masks (overlap the data DMA) ---
            aux_t = const.tile([128, 8], F32, tag="aux")
            nc.sync.dma_start(aux_t[:], aux_d[:])

            iota16 = const.tile([128, BW], F16, tag="iota16")
            nc.gpsimd.iota(iota16[:], pattern=[[1, BW]], base=0,
                           channel_multiplier=0,
                           allow_small_or_imprecise_dtypes=True)

            eye_src = const.tile([128, 128], F16, tag="eye_src")
            nc.gpsimd.memset(eye_src[:], 1.0)
            eye = const.tile([128, 128], F16, tag="eye")
            nc.gpsimd.affine_select(eye[:], eye_src[:], pattern=[[1, 128]],
                                    compare_op=ALU.is_equal, fill=0.0,
                                    base=0, channel_multiplier=-1)

            ones_col = const.tile([128, 1], F16, tag="ones_col")
            nc.vector.memset(ones_col[:], 1.0)
            ones_row = const.tile([1, 128], F16, tag="ones_row")
            nc.vector.memset(ones_row[:], 1.0)

            stats_t = const.tile([128, 4], F32, tag="stats")

            # chi masks: j < hi_i per strip (depend only on targets metadata)
            chi = []
            for s in range(NSTRIP):
                ch = const.tile([128, BW], F16, tag=f"chi{s}", name=f"chi{s}")
                nc.vector.tensor_scalar(out=ch[:], in0=iota16[:],
                                        scalar1=aux_t[:, s:s + 1],
                                        scalar2=None, op0=ALU.is_lt)
                chi.append(ch)

            # --- window DMA (4 chunks of 128 D-rows, split in halves) ---
            xt_c = []
            for k in range(NCHUNK):
                xk = const.tile([128, W], F16, tag=f"xt{k}", name=f"xt{k}")
                for h in range(2):
                    nc.sync.dma_start(xk[:, W // 2 * h:W // 2 * (h + 1)],
                                      xt_d[128 * k:128 * (k + 1),
                                           W // 2 * h:W // 2 * (h + 1)])
                xt_c.append(xk)

            # --- per-column sum of squares, column-major [128, 5] layout ---
            # g-major: one PSUM accumulation group at a time per zero-region
            ssT = psA.tile([128, 5], F32, tag="ssT")
            sqs = []
            for k in range(NCHUNK):
                sq = work.tile([128, W], F16, tag=f"sq{k}", name=f"sq{k}",
                               bufs=1)
                nc.vector.tensor_tensor(sq[:], xt_c[k][:], xt_c[k][:], ALU.mult)
                sqs.append(sq)
            for g in range(5):
                for k in range(NCHUNK):
                    nc.tensor.matmul(ssT[:, g:g + 1],
                                     sqs[k][:, 128 * g:128 * (g + 1)],
                                     ones_col[:, 0:1],
                                     start=(k == 0), stop=(k == NCHUNK - 1))

            # --- inv = 1/max(sqrt(ss), eps): row layout [128, 5] ---
            nrm = const.tile([128, 5], F32, tag="nrm")
            nc.scalar.activation(nrm[:], ssT[:], AF.Sqrt)
            nrmx = const.tile([128, 5], F32, tag="nrmx")
            nc.vector.tensor_scalar(out=nrmx[:], in0=nrm[:], scalar1=1e-8,
                                    scalar2=None, op0=ALU.max)
            invT = const.tile([128, 5], F32, tag="invT")
            nc.vector.reciprocal(invT[:], nrmx[:])
            invT16 = const.tile([128, 5], F16, tag="invT16")
            nc.scalar.activation(invT16[:], invT[:], AF.Copy)

            # --- broadcast inv along partitions ---
            # flatten invT16 [128, 5] -> [1, 640] via identity matmuls
            # (out[0, n] = invT16[n, g]), then gpsimd partition-broadcast.
            flatA = psA.tile([1, 512], F32, tag="flatA")
            flatB = psA.tile([1, 128], F32, tag="flatB")
            for g in range(5):
                dst = flatA[0:1, 128 * g:128 * (g + 1)] if g < 4 else flatB[0:1, :]
                nc.tensor.matmul(dst, invT16[:, g:g + 1], eye[:],
                                 start=True, stop=True)
            flat16 = const.tile([1, W], F16, tag="flat16")
            nc.vector.tensor_copy(flat16[0:1, 0:512], flatA[0:1, :])
            nc.vector.tensor_copy(flat16[0:1, 512:W], flatB[0:1, :])
            invBa = psA.tile([128, 512], F32, tag="invBa")
            invBb = psA.tile([128, 128], F32, tag="invBb")
            nc.tensor.matmul(invBa[:], ones_row[0:1, :], flat16[0:1, 0:512],
                             start=True, stop=True)
            nc.tensor.matmul(invBb[:], ones_row[0:1, :], flat16[0:1, 512:W],
                             start=True, stop=True)
            invB16 = const.tile([128, W], F16, tag="invB16")
            nc.scalar.activation(invB16[:, 0:512], invBa[:], AF.Copy)
            nc.scalar.activation(invB16[:, 512:W], invBb[:], AF.Copy)

            # --- banded Gram + masked epilogue per 128-row strip ---
            for s in range(NSTRIP):
                G = psG.tile([128, BW], F32, tag="g")
                for k in range(NCHUNK):
                    nc.tensor.matmul(G[:],
                                     xt_c[k][:, 128 * s:128 * s + 128],
                                     xt_c[k][:, 128 * s:128 * s + BW],
                                     start=(k == 0), stop=(k == NCHUNK - 1))
                t1 = work.tile([128, BW], F16, tag="t1")
                nc.vector.tensor_tensor(t1[:], G[:],
                                        invB16[:, 128 * s:128 * s + BW],
                                        ALU.mult)
                u2 = work.tile([128, BW], F16, tag="u2")
                nc.scalar.activation(u2[:], t1[:], AF.Copy, bias=-1.0,
                                     scale=invT[:, s:s + 1])
                v = work.tile([128, BW], F16, tag="v")
                nc.gpsimd.tensor_tensor(v[:], u2[:], chi[s][:], ALU.mult)
                v2 = work.tile([128, BW], F16, tag="v2")
                nc.gpsimd.affine_select(v2[:], v[:], pattern=[[1, BW]],
                                        compare_op=ALU.is_ge, fill=0.0,
                                        base=-1, channel_multiplier=-1)
                junk = work.tile([128, BW], F16, tag="junk")
                nc.scalar.activation(junk[:], v2[:], AF.Square,
                                     accum_out=stats_t[:, s:s + 1])

            nc.sync.dma_start(stats_d[:], stats_t[:])
    nc.finalize()
    return nc


def host_prepare(inputs, targets):
    """Sort rows by label, build per-core transposed windows + band bounds."""
    inputs = np.asarray(inputs, np.float32)
    targets_np = np.asarray(targets)
    order = np.argsort(targets_np, kind="stable")
    ts = targets_np[order]
    X16 = inputs[order].astype(np.float16)
    hi = np.searchsorted(ts, ts, side="right").astype(np.int64)
    idx = np.arange(N)
    bmax = int((hi - idx).max())
    if bmax > BW - 127:
        raise NotImplementedError(
            f"label block overhang {bmax} exceeds supported band ({BW - 127})")

    XT = np.ascontiguousarray(X16.T)  # [D, N]

    in_maps = []
    for c in range(NCORES):
        cols = (RPC * c + np.arange(W)) % N
        xt_c = np.ascontiguousarray(XT[:, cols])
        aux = np.zeros((128, 8), np.float32)
        for s in range(NSTRIP):
            base = RPC * c + 128 * s
            aux[:, s] = (hi[base:base + 128] - base).astype(np.float32)
        in_maps.append({"xt": xt_c, "aux": aux})

    cnts = np.bincount(targets_np.astype(np.int64))
    pos_cnt = float((cnts * (cnts - 1) // 2).sum())
    neg_cnt = float(N * N - (cnts * cnts).sum())
    return in_maps, pos_cnt, neg_cnt


def combine(stats_list, pos_cnt, neg_cnt):
    pos_sum = 0.0
    for st in stats_list:
        pos_sum += np.asarray(st, np.float64).sum()
    # neg pairs all have cos_sim < 1 => relu(cos_sim - margin) == 0 exactly
    loss = np.float32(pos_sum / pos_cnt + 0.0 / neg_cnt)
    return np.asarray(loss, np.float32)


_prog_cache = {}


def kernel(inputs, targets):
    from concourse.bass_utils import run_bass_kernel_spmd
    in_maps, pos_cnt, neg_cnt = host_prepare(inputs, targets)
    if "nc" not in _prog_cache:
        _prog_cache["nc"] = build_program()
    nc = _prog_cache["nc"]
    res = run_bass_kernel_spmd(nc, in_maps, list(range(NCORES)))
    stats_list = [res.results[c]["stats"] for c in range(NCORES)]
    return combine(stats_list, pos_cnt, neg_cnt)


# revision 10
# speedup vs baseline: 3.2495x; 1.1411x over previous
"""Contrastive loss kernel for Trainium2 (8 NeuronCores, SPMD).

Math: loss = mean_{pos pairs}(1-cos_sim)^2 + mean_{neg pairs}relu(cos_sim-1)^2
with pos = same-label upper-triangle pairs, neg = different-label ordered pairs.

Since cos_sim(x_i, x_j) <= 1 for all pairs (Cauchy-Schwarz on normalized
vectors, strict for non-parallel vectors), relu(cos_sim-1) is identically zero
on every neg pair, so the neg term contributes exactly 0/neg_cnt. Only the pos
term needs computing, and pos pairs are confined to same-label blocks.

Strategy:
  * Host sorts rows by label (stable), so same-label pairs form contiguous
    blocks along the diagonal. Each core owns 512 rows and computes, for each
    of its rows i, Gram entries for columns j in (i, hi_i) -- a 224-wide band
    per 128-row strip (supports label blocks up to 97 rows; actual max ~82).
  * One fat input param per core, [128, 3456] fp16 with 6912B contiguous
    partition lines: 4 transposed-window chunks (cols 640k..640k+640 hold
    D-rows 128k..128k+128 of the [512, 640] X^T window) + 4 precomputed
    [128, 224] pos-mask tiles. DMA split across both HWDGE queues
    (sync + scalar engines).
  * Norms: square the chunks (vector), reduce over D with 20 [128,128]x[128,1]
    ones-matmuls into a column-major [128, 5] PSUM tile, sqrt/clamp/reciprocal,
    then flatten via identity matmuls and broadcast along partitions with K=1
    ones-matmuls.
  * Gram: raw fp16 band matmuls (4 strips x 4 K-chunks, [128x128]x[128x224]).
  * Epilogue per strip (vector): s = G * inv_j, then (s * inv_i - 1) * mask,
    then Square-accumulate on the scalar engine into per-partition partials.
  * Host sums the 8 x [128, 4] partials and divides by the exact pair count.
"""

import numpy as np

import concourse.bass as bass
import concourse.bacc as bacc
import concourse.mybir as mybir
import concourse.tile as tile

N, D, NCORES = 4096, 512, 8
RPC = N // NCORES   # 512 rows per core
W = 640             # window columns per core
BW = 224            # band width per 128-row strip
NSTRIP = RPC // 128
NCHUNK = D // 128
CH_COLS = NCHUNK * W          # 2560: chunk region of the fat param
TOT_COLS = CH_COLS + NSTRIP * BW  # 3456: + mask region

F32 = mybir.dt.float32
F16 = mybir.dt.float16
AF = mybir.ActivationFunctionType
ALU = mybir.AluOpType


def build_program():
    nc = bacc.Bacc(None)
    xt_d = nc.declare_dram_parameter("xt", [128, TOT_COLS], F16, isOutput=False)
    stats_d = nc.declare_dram_parameter("stats", [128, 4], F32, isOutput=True)

    with tile.TileContext(nc) as tc:
        with (
            tc.tile_pool(name="const", bufs=1) as const,
            tc.tile_pool(name="work", bufs=2) as work,
            tc.tile_pool(name="psA", bufs=1, space="PSUM") as psA,
            tc.tile_pool(name="psG", bufs=3, space="PSUM") as psG,
        ):
            # --- fat input DMA, split across both HWDGE queues ---
            xt = const.tile([128, TOT_COLS], F16, tag="xt")
            HALF = TOT_COLS // 2
            nc.sync.dma_start(xt[:, 0:HALF], xt_d[:, 0:HALF])
            nc.scalar.dma_start(xt[:, HALF:TOT_COLS], xt_d[:, HALF:TOT_COLS])

            def chunk(k):
                return xt[:, W * k:W * (k + 1)]

            def mask(s):
                return xt[:, CH_COLS + BW * s:CH_COLS + BW * (s + 1)]

            # identity for the flatten matmuls (gpsimd, off critical path)
            eye_src = const.tile([128, 128], F16, tag="eye_src")
            nc.gpsimd.memset(eye_src[:], 1.0)
            eye = const.tile([128, 128], F16, tag="eye")
            nc.gpsimd.affine_select(eye[:], eye_src[:], pattern=[[1, 128]],
                                    compare_op=ALU.is_equal, fill=0.0,
                                    base=0, channel_multiplier=-1)

            ones_col = const.tile([128, 1], F16, tag="ones_col")
            nc.vector.memset(ones_col[:], 1.0)
            ones_row = const.tile([1, 128], F16, tag="ones_row")
            nc.vector.memset(ones_row[:], 1.0)

            stats_t = const.tile([128, 4], F32, tag="stats")

            # --- squares of the chunk region (two halves) ---
            sq = const.tile([128, CH_COLS], F16, tag="sq")
            nc.vector.tensor_tensor(sq[:, 0:CH_COLS // 2], xt[:, 0:CH_COLS // 2],
                                    xt[:, 0:CH_COLS // 2], ALU.mult)
            nc.vector.tensor_tensor(sq[:, CH_COLS // 2:CH_COLS],
                                    xt[:, CH_COLS // 2:CH_COLS],
                                    xt[:, CH_COLS // 2:CH_COLS], ALU.mult)

            # --- per-column sum of squares, column-major [128, 5] ---
            ssT = psA.tile([128, 5], F32, tag="ssT")
            for g in range(5):
                for k in range(NCHUNK):
                    nc.tensor.matmul(ssT[:, g:g + 1],
                                     sq[:, W * k + 128 * g:W * k + 128 * (g + 1)],
                                     ones_col[:, 0:1],
                                     start=(k == 0), stop=(k == NCHUNK - 1))

            # --- inv = 1/max(sqrt(ss), eps) ---
            nrm = const.tile([128, 5], F32, tag="nrm")
            nc.scalar.activation(nrm[:], ssT[:], AF.Sqrt)
            nrmx = const.tile([128, 5], F32, tag="nrmx")
            nc.vector.tensor_scalar(out=nrmx[:], in0=nrm[:], scalar1=1e-8,
                                    scalar2=None, op0=ALU.max)
            invT = const.tile([128, 5], F32, tag="invT")
            nc.vector.reciprocal(invT[:], nrmx[:])
            invT16 = const.tile([128, 5], F16, tag="invT16")
            nc.vector.tensor_copy(invT16[:], invT[:])

            # --- flatten [128, 5] -> [1, 640] via identity matmuls ---
            flatA = psA.tile([1, 512], F32, tag="flatA")
            flatB = psA.tile([1, 128], F32, tag="flatB")
            for g in range(5):
                dst = flatA[0:1, 128 * g:128 * (g + 1)] if g < 4 else flatB[0:1, :]
                nc.tensor.matmul(dst, invT16[:, g:g + 1], eye[:],
                                 start=True, stop=True)
            flat16 = const.tile([1, W], F16, tag="flat16")
            nc.vector.tensor_copy(flat16[0:1, 0:512], flatA[0:1, :])
            nc.vector.tensor_copy(flat16[0:1, 512:W], flatB[0:1, :])

            # --- broadcast along partitions via K=1 ones-matmuls ---
            invBa = psA.tile([128, 512], F32, tag="invBa")
            invBb = psA.tile([128, 128], F32, tag="invBb")
            nc.tensor.matmul(invBa[:], ones_row[0:1, :], flat16[0:1, 0:512],
                             start=True, stop=True)
            nc.tensor.matmul(invBb[:], ones_row[0:1, :], flat16[0:1, 512:W],
                             start=True, stop=True)
            invB16 = const.tile([128, W], F16, tag="invB16")
            nc.vector.tensor_copy(invB16[:, 0:512], invBa[:])
            nc.vector.tensor_copy(invB16[:, 512:W], invBb[:])

            # --- banded Gram + masked epilogue per 128-row strip ---
            for s in range(NSTRIP):
                G = psG.tile([128, BW], F32, tag="g")
                for k in range(NCHUNK):
                    nc.tensor.matmul(G[:],
                                     xt[:, W * k + 128 * s:W * k + 128 * s + 128],
                                     xt[:, W * k + 128 * s:W * k + 128 * s + BW],
                                     start=(k == 0), stop=(k == NCHUNK - 1))
                t1 = work.tile([128, BW], F16, tag="t1")
                nc.vector.tensor_tensor(t1[:], G[:],
                                        invB16[:, 128 * s:128 * s + BW],
                                        ALU.mult)
                u2 = work.tile([128, BW], F16, tag="u2")
                nc.vector.tensor_scalar(out=u2[:], in0=t1[:],
                                        scalar1=invT[:, s:s + 1], scalar2=1.0,
                                        op0=ALU.mult, op1=ALU.subtract)
                v = work.tile([128, BW], F16, tag="v")
                nc.vector.tensor_tensor(v[:], u2[:], mask(s), ALU.mult)
                junk = work.tile([128, BW], F16, tag="junk")
                nc.scalar.activation(junk[:], v[:], AF.Square,
                                     accum_out=stats_t[:, s:s + 1])

            nc.sync.dma_start(stats_d[:], stats_t[:])
    nc.finalize()
    return nc


def host_prepare(inputs, targets):
    """Sort rows by label, build per-core fat params (windows + masks)."""
    inputs = np.asarray(inputs, np.float32)
    targets_np = np.asarray(targets)
    order = np.argsort(targets_np, kind="stable")
    ts = targets_np[order]
    X16 = inputs[order].astype(np.float16)
    hi = np.searchsorted(ts, ts, side="right").astype(np.int64)
    idx = np.arange(N)
    bmax = int((hi - idx).max())
    if bmax > BW - 127:
        raise NotImplementedError(
            f"label block overhang {bmax} exceeds supported band ({BW - 127})")

    XT = np.ascontiguousarray(X16.T)  # [D, N]
    jj = np.arange(BW)[None, :]
    pp = np.arange(128)[:, None]

    in_maps = []
    for c in range(NCORES):
        cols = (RPC * c + np.arange(W)) % N
        win = XT[:, cols]                              # [512, 640]
        fat = np.empty((128, TOT_COLS), np.float16)
        fat[:, 0:CH_COLS] = (
            win.reshape(NCHUNK, 128, W).transpose(1, 0, 2).reshape(128, CH_COLS))
        for s in range(NSTRIP):
            base = RPC * c + 128 * s
            hib = (hi[base:base + 128] - base)[:, None]  # band-relative hi
            pu = (jj > pp) & (jj < hib)
            fat[:, CH_COLS + BW * s:CH_COLS + BW * (s + 1)] = pu
        in_maps.append({"xt": fat})

    cnts = np.bincount(targets_np.astype(np.int64))
    pos_cnt = float((cnts * (cnts - 1) // 2).sum())
    neg_cnt = float(N * N - (cnts * cnts).sum())
    return in_maps, pos_cnt, neg_cnt


def combine(stats_list, pos_cnt, neg_cnt):
    pos_sum = 0.0
    for st in stats_list:
        pos_sum += np.asarray(st, np.float64).sum()
    # neg pairs all have cos_sim < 1 => relu(cos_sim - margin) == 0 exactly
    loss = np.float32(pos_sum / pos_cnt + 0.0 / neg_cnt)
    return np.asarray(loss, np.float32)


_prog_cache = {}


def kernel(inputs, targets):
    from concourse.bass_utils import run_bass_kernel_spmd
    in_maps, pos_cnt, neg_cnt = host_prepare(inputs, targets)
    if "nc" not in _prog_cache:
        _prog_cache["nc"] = build_program()
    nc = _prog_cache["nc"]
    res = run_bass_kernel_spmd(nc, in_maps, list(range(NCORES)))
    stats_list = [res.results[c]["stats"] for c in range(NCORES)]
    return combine(stats_list, pos_cnt, neg_cnt)


# revision 11
# speedup vs baseline: 3.3220x; 1.0223x over previous
"""Contrastive loss kernel for Trainium2 (8 NeuronCores, SPMD).

Math: loss = mean_{pos pairs}(1-cos_sim)^2 + mean_{neg pairs}relu(cos_sim-1)^2
with pos = same-label upper-triangle pairs, neg = different-label ordered pairs.

Since cos_sim(x_i, x_j) <= 1 for all pairs (Cauchy-Schwarz on normalized
vectors, strict for non-parallel vectors), relu(cos_sim-1) is identically zero
on every neg pair, so the neg term contributes exactly 0/neg_cnt. Only the pos
term needs computing, and pos pairs are confined to same-label blocks.

Strategy:
  * Host sorts rows by label (stable), so same-label pairs form contiguous
    blocks along the diagonal. Each core owns 512 rows and computes, for each
    of its rows i, Gram entries for columns j in (i, hi_i) -- a 224-wide band
    per 128-row strip (supports label blocks up to 97 rows; actual max ~82).
  * One fat input param per core, [128, 3456] fp16 with 6912B contiguous
    partition lines: 4 transposed-window chunks (cols 640k..640k+640 hold
    D-rows 128k..128k+128 of the [512, 640] X^T window) + 4 precomputed
    [128, 224] pos-mask tiles. DMA split across both HWDGE queues
    (sync + scalar engines).
  * Norms: square the chunks (vector), reduce over D with 20 [128,128]x[128,1]
    ones-matmuls into a column-major [128, 5] PSUM tile, sqrt/clamp/reciprocal,
    then flatten via identity matmuls and broadcast along partitions with K=1
    ones-matmuls.
  * Gram: raw fp16 band matmuls (4 strips x 4 K-chunks, [128x128]x[128x224]).
  * Epilogue per strip (vector): s = G * inv_j, then (s * inv_i - 1) * mask,
    then Square-accumulate on the scalar engine into per-partition partials.
  * Host sums the 8 x [128, 4] partials and divides by the exact pair count.
"""

import numpy as np

import concourse.bass as bass
import concourse.bacc as bacc
import concourse.mybir as mybir
import concourse.tile as tile

N, D, NCORES = 4096, 512, 8
RPC = N // NCORES   # 512 rows per core
W = 640             # window columns per core
BW = 224            # band width per 128-row strip
NSTRIP = RPC // 128
NCHUNK = D // 128
CH_COLS = NCHUNK * W          # 2560: chunk region of the fat param
TOT_COLS = CH_COLS + NSTRIP * BW  # 3456: + mask region

F32 = mybir.dt.float32
F16 = mybir.dt.float16
AF = mybir.ActivationFunctionType
ALU = mybir.AluOpType


def build_program():
    nc = bacc.Bacc(None)
    xt_d = nc.declare_dram_parameter("xt", [128, TOT_COLS], F16, isOutput=False)
    stats_d = nc.declare_dram_parameter("stats", [128, 4], F32, isOutput=True)

    with tile.TileContext(nc) as tc:
        with (
            tc.tile_pool(name="const", bufs=1) as const,
            tc.tile_pool(name="work", bufs=2) as work,
            tc.tile_pool(name="psA", bufs=1, space="PSUM") as psA,
            tc.tile_pool(name="psG", bufs=3, space="PSUM") as psG,
        ):
            # --- fat input DMA, split across both HWDGE queues ---
            xt = const.tile([128, TOT_COLS], F16, tag="xt")
            HALF = TOT_COLS // 2
            nc.scalar.dma_start(xt[:, 0:HALF], xt_d[:, 0:HALF])
            nc.scalar.dma_start(xt[:, HALF:TOT_COLS], xt_d[:, HALF:TOT_COLS])

            def chunk(k):
                return xt[:, W * k:W * (k + 1)]

            def mask(s):
                return xt[:, CH_COLS + BW * s:CH_COLS + BW * (s + 1)]

            # identity for the flatten matmuls (gpsimd, off critical path)
            eye_src = const.tile([128, 128], F16, tag="eye_src")
            nc.gpsimd.memset(eye_src[:], 1.0)
            eye = const.tile([128, 128], F16, tag="eye")
            nc.gpsimd.affine_select(eye[:], eye_src[:], pattern=[[1, 128]],
                                    compare_op=ALU.is_equal, fill=0.0,
                                    base=0, channel_multiplier=-1)

            ones_col = const.tile([128, 1], F16, tag="ones_col")
            nc.vector.memset(ones_col[:], 1.0)
            ones_row = const.tile([1, 128], F16, tag="ones_row")
            nc.vector.memset(ones_row[:], 1.0)

            stats_t = const.tile([128, 4], F32, tag="stats")

            # --- squares of the chunk region (two halves) ---
            sq = const.tile([128, CH_COLS], F16, tag="sq")
            nc.vector.tensor_tensor(sq[:, 0:CH_COLS // 2], xt[:, 0:CH_COLS // 2],
                                    xt[:, 0:CH_COLS // 2], ALU.mult)
            nc.vector.tensor_tensor(sq[:, CH_COLS // 2:CH_COLS],
                                    xt[:, CH_COLS // 2:CH_COLS],
                                    xt[:, CH_COLS // 2:CH_COLS], ALU.mult)

            # --- per-column sum of squares, column-major [128, 5] ---
            ssT = psA.tile([128, 5], F32, tag="ssT")
            for g in range(5):
                for k in range(NCHUNK):
                    nc.tensor.matmul(ssT[:, g:g + 1],
                                     sq[:, W * k + 128 * g:W * k + 128 * (g + 1)],
                                     ones_col[:, 0:1],
                                     start=(k == 0), stop=(k == NCHUNK - 1))

            # --- inv = 1/max(sqrt(ss), eps) ---
            nrm = const.tile([128, 5], F32, tag="nrm")
            nc.scalar.activation(nrm[:], ssT[:], AF.Sqrt)
            nrmx = const.tile([128, 5], F32, tag="nrmx")
            nc.vector.tensor_scalar(out=nrmx[:], in0=nrm[:], scalar1=1e-8,
                                    scalar2=None, op0=ALU.max)
            invT = const.tile([128, 5], F32, tag="invT")
            nc.vector.reciprocal(invT[:], nrmx[:])
            invT16 = const.tile([128, 5], F16, tag="invT16")
            nc.vector.tensor_copy(invT16[:], invT[:])

            # --- flatten [128, 5] -> [1, 640] via identity matmuls ---
            flatA = psA.tile([1, 512], F32, tag="flatA")
            flatB = psA.tile([1, 128], F32, tag="flatB")
            for g in range(5):
                dst = flatA[0:1, 128 * g:128 * (g + 1)] if g < 4 else flatB[0:1, :]
                nc.tensor.matmul(dst, invT16[:, g:g + 1], eye[:],
                                 start=True, stop=True)
            flat16 = const.tile([1, W], F16, tag="flat16")
            nc.vector.tensor_copy(flat16[0:1, 0:512], flatA[0:1, :])
            nc.vector.tensor_copy(flat16[0:1, 512:W], flatB[0:1, :])

            # --- broadcast along partitions via K=1 ones-matmuls ---
            invBa = psA.tile([128, 512], F32, tag="invBa")
            invBb = psA.tile([128, 128], F32, tag="invBb")
            nc.tensor.matmul(invBa[:], ones_row[0:1, :], flat16[0:1, 0:512],
                             start=True, stop=True)
            nc.tensor.matmul(invBb[:], ones_row[0:1, :], flat16[0:1, 512:W],
                             start=True, stop=True)
            invB16 = const.tile([128, W], F16, tag="invB16")
            nc.vector.tensor_copy(invB16[:, 0:512], invBa[:])
            nc.vector.tensor_copy(invB16[:, 512:W], invBb[:])

            # --- banded Gram + masked epilogue per 128-row strip ---
            for s in range(NSTRIP):
                G = psG.tile([128, BW], F32, tag="g")
                for k in range(NCHUNK):
                    nc.tensor.matmul(G[:],
                                     xt[:, W * k + 128 * s:W * k + 128 * s + 128],
                                     xt[:, W * k + 128 * s:W * k + 128 * s + BW],
                                     start=(k == 0), stop=(k == NCHUNK - 1))
                t1 = work.tile([128, BW], F16, tag="t1")
                nc.vector.tensor_tensor(t1[:], G[:],
                                        invB16[:, 128 * s:128 * s + BW],
                                        ALU.mult)
                u2 = work.tile([128, BW], F16, tag="u2")
                nc.vector.tensor_scalar(out=u2[:], in0=t1[:],
                                        scalar1=invT[:, s:s + 1], scalar2=1.0,
                                        op0=ALU.mult, op1=ALU.subtract)
                v = work.tile([128, BW], F16, tag="v")
                nc.vector.tensor_tensor(v[:], u2[:], mask(s), ALU.mult)
                junk = work.tile([128, BW], F16, tag="junk")
                nc.scalar.activation(junk[:], v[:], AF.Square,
                                     accum_out=stats_t[:, s:s + 1])

            nc.sync.dma_start(stats_d[:], stats_t[:])
    nc.finalize()
    return nc


def host_prepare(inputs, targets):
    """Sort rows by label, build per-core fat params (windows + masks)."""
    inputs = np.asarray(inputs, np.float32)
    targets_np = np.asarray(targets)
    order = np.argsort(targets_np, kind="stable")
    ts = targets_np[order]
    X16 = inputs[order].astype(np.float16)
    hi = np.searchsorted(ts, ts, side="right").astype(np.int64)
    idx = np.arange(N)
    bmax = int((hi - idx).max())
    if bmax > BW - 127:
        raise NotImplementedError(
            f"label block overhang {bmax} exceeds supported band ({BW - 127})")

    XT = np.ascontiguousarray(X16.T)  # [D, N]
    jj = np.arange(BW)[None, :]
    pp = np.arange(128)[:, None]

    in_maps = []
    for c in range(NCORES):
        cols = (RPC * c + np.arange(W)) % N
        win = XT[:, cols]                              # [512, 640]
        fat = np.empty((128, TOT_COLS), np.float16)
        fat[:, 0:CH_COLS] = (
            win.reshape(NCHUNK, 128, W).transpose(1, 0, 2).reshape(128, CH_COLS))
        for s in range(NSTRIP):
            base = RPC * c + 128 * s
            hib = (hi[base:base + 128] - base)[:, None]  # band-relative hi
            pu = (jj > pp) & (jj < hib)
            fat[:, CH_COLS + BW * s:CH_COLS + BW * (s + 1)] = pu
        in_maps.append({"xt": fat})

    cnts = np.bincount(targets_np.astype(np.int64))
    pos_cnt = float((cnts * (cnts - 1) // 2).sum())
    neg_cnt = float(N * N - (cnts * cnts).sum())
    return in_maps, pos_cnt, neg_cnt


def combine(stats_list, pos_cnt, neg_cnt):
    pos_sum = 0.0
    for st in stats_list:
        pos_sum += np.asarray(st, np.float64).sum()
    # neg pairs all have cos_sim < 1 => relu(cos_sim - margin) == 0 exactly
    loss = np.float32(pos_sum / pos_cnt + 0.0 / neg_cnt)
    return np.asarray(loss, np.float32)


_prog_cache = {}


def kernel(inputs, targets):
    from concourse.bass_utils import run_bass_kernel_spmd
    in_maps, pos_cnt, neg_cnt = host_prepare(inputs, targets)
    if "nc" not in _prog_cache:
        _prog_cache["nc"] = build_program()
    nc = _prog_cache["nc"]
    res = run_bass_kernel_spmd(nc, in_maps, list(range(NCORES)))
    stats_list = [res.results[c]["stats"] for c in range(NCORES)]
    return combine(stats_list, pos_cnt, neg_cnt)


# revision 13
# speedup vs baseline: 3.5671x; 1.0738x over previous
"""Contrastive loss kernel for Trainium2 (8 NeuronCores, SPMD).

Math: loss = mean_{pos pairs}(1-cos_sim)^2 + mean_{neg pairs}relu(cos_sim-1)^2
with pos = same-label upper-triangle pairs, neg = different-label ordered pairs.

Since cos_sim(x_i, x_j) <= 1 for all pairs (Cauchy-Schwarz on normalized
vectors, strict for non-parallel vectors), relu(cos_sim-1) is identically zero
on every neg pair, so the neg term contributes exactly 0/neg_cnt. Only the pos
term needs computing, and pos pairs are confined to same-label blocks.

Strategy:
  * Host sorts rows by label (stable), so same-label pairs form contiguous
    blocks along the diagonal. Each core owns 512 rows and computes, for each
    of its rows i, Gram entries for columns j in (i, hi_i) -- a 224-wide band
    per 128-row strip (supports label blocks up to 97 rows; actual max ~82).
  * One fat input param per core, [128, 3456] fp16 with 6912B contiguous
    partition lines: 4 transposed-window chunks (cols 640k..640k+640 hold
    D-rows 128k..128k+128 of the [512, 640] X^T window) + 4 precomputed
    [128, 224] pos-mask tiles. DMA split across both HWDGE queues
    (sync + scalar engines).
  * Norms: square the chunks (vector), reduce over D with 20 [128,128]x[128,1]
    ones-matmuls into a column-major [128, 5] PSUM tile, sqrt/clamp/reciprocal,
    then flatten via identity matmuls and broadcast along partitions with K=1
    ones-matmuls.
  * Gram: raw fp16 band matmuls (4 strips x 4 K-chunks, [128x128]x[128x224]).
  * Epilogue per strip (vector): s = G * inv_j, then (s * inv_i - 1) * mask,
    then Square-accumulate on the scalar engine into per-partition partials.
  * Host sums the 8 x [128, 4] partials and divides by the exact pair count.
"""

import numpy as np

import concourse.bass as bass
import concourse.bacc as bacc
import concourse.mybir as mybir
import concourse.tile as tile

N, D, NCORES = 4096, 512, 8
RPC = N // NCORES   # 512 rows per core
W = 640             # window columns per core
BW = 224            # band width per 128-row strip
NSTRIP = RPC // 128
NCHUNK = D // 128
CH_COLS = NCHUNK * W          # 2560: chunk region of the fat param
TOT_COLS = CH_COLS + NSTRIP * BW  # 3456: + mask region

F32 = mybir.dt.float32
F16 = mybir.dt.float16
AF = mybir.ActivationFunctionType
ALU = mybir.AluOpType


def build_program():
    nc = bacc.Bacc(None)
    xt_d = nc.declare_dram_parameter("xt", [128, TOT_COLS], F16, isOutput=False)
    stats_d = nc.declare_dram_parameter("stats", [128, 4], F32, isOutput=True)

    with tile.TileContext(nc) as tc:
        with (
            tc.tile_pool(name="const", bufs=1) as const,
            tc.tile_pool(name="work", bufs=2) as work,
            tc.tile_pool(name="psA", bufs=1, space="PSUM") as psA,
            tc.tile_pool(name="psG", bufs=3, space="PSUM") as psG,
        ):
            # --- fat input DMA, split across both HWDGE queues ---
            xt = const.tile([128, TOT_COLS], F16, tag="xt")
            HALF = TOT_COLS // 2
            nc.scalar.dma_start(xt[:, 0:HALF], xt_d[:, 0:HALF])
            nc.scalar.dma_start(xt[:, HALF:TOT_COLS], xt_d[:, HALF:TOT_COLS])

            def chunk(k):
                return xt[:, W * k:W * (k + 1)]

            def mask(s):
                return xt[:, CH_COLS + BW * s:CH_COLS + BW * (s + 1)]

            # identity for the flatten matmuls (gpsimd, off critical path)
            eye_src = const.tile([128, 128], F16, tag="eye_src")
            nc.gpsimd.memset(eye_src[:], 1.0)
            eye = const.tile([128, 128], F16, tag="eye")
            nc.gpsimd.affine_select(eye[:], eye_src[:], pattern=[[1, 128]],
                                    compare_op=ALU.is_equal, fill=0.0,
                                    base=0, channel_multiplier=-1)

            ones_col = const.tile([128, 1], F16, tag="ones_col")
            nc.vector.memset(ones_col[:], 1.0)
            ones_row = const.tile([1, 128], F16, tag="ones_row")
            nc.vector.memset(ones_row[:], 1.0)

            stats_t = const.tile([128, 4], F32, tag="stats")

            # --- squares of the chunk region (two halves) ---
            sq = const.tile([128, CH_COLS], F16, tag="sq")
            nc.vector.tensor_tensor(sq[:, 0:CH_COLS // 2], xt[:, 0:CH_COLS // 2],
                                    xt[:, 0:CH_COLS // 2], ALU.mult)
            nc.vector.tensor_tensor(sq[:, CH_COLS // 2:CH_COLS],
                                    xt[:, CH_COLS // 2:CH_COLS],
                                    xt[:, CH_COLS // 2:CH_COLS], ALU.mult)

            # --- per-column sum of squares, column-major [128, 5] ---
            ssT = psA.tile([128, 5], F32, tag="ssT")
            for g in range(5):
                for k in range(NCHUNK):
                    nc.tensor.matmul(ssT[:, g:g + 1],
                                     sq[:, W * k + 128 * g:W * k + 128 * (g + 1)],
                                     ones_col[:, 0:1],
                                     start=(k == 0), stop=(k == NCHUNK - 1))

            # --- inv = 1/max(sqrt(ss), eps) ---
            nrm = const.tile([128, 5], F32, tag="nrm")
            nc.scalar.activation(nrm[:], ssT[:], AF.Sqrt)
            nrmx = const.tile([128, 5], F32, tag="nrmx")
            nc.vector.tensor_scalar(out=nrmx[:], in0=nrm[:], scalar1=1e-8,
                                    scalar2=None, op0=ALU.max)
            invT = const.tile([128, 5], F32, tag="invT")
            nc.vector.reciprocal(invT[:], nrmx[:])
            invT16 = const.tile([128, 5], F16, tag="invT16")
            nc.vector.tensor_copy(invT16[:], invT[:])

            # --- banded Gram matmuls; strip 0 first so PE stays busy while
            # the vector engine finishes the reciprocal chain ---
            def gram(s):
                G = psG.tile([128, BW], F32, tag="g", name=f"g{s}")
                for k in range(NCHUNK):
                    nc.tensor.matmul(G[:],
                                     xt[:, W * k + 128 * s:W * k + 128 * s + 128],
                                     xt[:, W * k + 128 * s:W * k + 128 * s + BW],
                                     start=(k == 0), stop=(k == NCHUNK - 1))
                return G

            G0 = gram(0)

            # --- flatten [128, 5] -> [1, 640] via identity matmuls ---
            flatA = psA.tile([1, 512], F32, tag="flatA")
            flatB = psA.tile([1, 128], F32, tag="flatB")
            for g in range(5):
                dst = flatA[0:1, 128 * g:128 * (g + 1)] if g < 4 else flatB[0:1, :]
                nc.tensor.matmul(dst, invT16[:, g:g + 1], eye[:],
                                 start=True, stop=True)
            flat16 = const.tile([1, W], F16, tag="flat16")
            nc.vector.tensor_copy(flat16[0:1, 0:512], flatA[0:1, :])
            nc.vector.tensor_copy(flat16[0:1, 512:W], flatB[0:1, :])

            # --- broadcast along partitions via K=1 ones-matmuls ---
            invBa = psA.tile([128, 512], F32, tag="invBa")
            invBb = psA.tile([128, 128], F32, tag="invBb")
            nc.tensor.matmul(invBa[:], ones_row[0:1, :], flat16[0:1, 0:512],
                             start=True, stop=True)
            nc.tensor.matmul(invBb[:], ones_row[0:1, :], flat16[0:1, 512:W],
                             start=True, stop=True)
            invB16 = const.tile([128, W], F16, tag="invB16")
            nc.vector.tensor_copy(invB16[:, 0:512], invBa[:])
            nc.vector.tensor_copy(invB16[:, 512:W], invBb[:])

            # --- remaining Gram strips + masked epilogue per strip ---
            def epilogue(s, G):
                t1 = work.tile([128, BW], F16, tag="t1", name=f"t1_{s}")
                nc.vector.tensor_tensor(t1[:], G[:],
                                        invB16[:, 128 * s:128 * s + BW],
                                        ALU.mult)
                u2 = work.tile([128, BW], F16, tag="u2", name=f"u2_{s}")
                nc.vector.tensor_scalar(out=u2[:], in0=t1[:],
                                        scalar1=invT[:, s:s + 1], scalar2=1.0,
                                        op0=ALU.mult, op1=ALU.subtract)
                v = work.tile([128, BW], F16, tag="v", name=f"v_{s}")
                nc.vector.tensor_tensor(v[:], u2[:], mask(s), ALU.mult)
                junk = work.tile([128, BW], F16, tag="junk", name=f"junk{s}")
                nc.scalar.activation(junk[:], v[:], AF.Square,
                                     accum_out=stats_t[:, s:s + 1])

            prev = G0
            for s in range(1, NSTRIP):
                Gn = gram(s)
                epilogue(s - 1, prev)
                prev = Gn
            epilogue(NSTRIP - 1, prev)

            nc.sync.dma_start(stats_d[:], stats_t[:])
    nc.finalize()
    return nc


def host_prepare(inputs, targets):
    """Sort rows by label, build per-core fat params (windows + masks)."""
    inputs = np.asarray(inputs, np.float32)
    targets_np = np.asarray(targets)
    order = np.argsort(targets_np, kind="stable")
    ts = targets_np[order]
    X16 = inputs[order].astype(np.float16)
    hi = np.searchsorted(ts, ts, side="right").astype(np.int64)
    idx = np.arange(N)
    bmax = int((hi - idx).max())
    if bmax > BW - 127:
        raise NotImplementedError(
            f"label block overhang {bmax} exceeds supported band ({BW - 127})")

    XT = np.ascontiguousarray(X16.T)  # [D, N]
    jj = np.arange(BW)[None, :]
    pp = np.arange(128)[:, None]

    in_maps = []
    for c in range(NCORES):
        cols = (RPC * c + np.arange(W)) % N
        win = XT[:, cols]                              # [512, 640]
        fat = np.empty((128, TOT_COLS), np.float16)
        fat[:, 0:CH_COLS] = (
            win.reshape(NCHUNK, 128, W).transpose(1, 0, 2).reshape(128, CH_COLS))
        for s in range(NSTRIP):
            base = RPC * c + 128 * s
            hib = (hi[base:base + 128] - base)[:, None]  # band-relative hi
            pu = (jj > pp) & (jj < hib)
            fat[:, CH_COLS + BW * s:CH_COLS + BW * (s + 1)] = pu
        in_maps.append({"xt": fat})

    cnts = np.bincount(targets_np.astype(np.int64))
    pos_cnt = float((cnts * (cnts - 1) // 2).sum())
    neg_cnt = float(N * N - (cnts * cnts).sum())
    return in_maps, pos_cnt, neg_cnt


def combine(stats_list, pos_cnt, neg_cnt):
    pos_sum = 0.0
    for st in stats_list:
        pos_sum += np.asarray(st, np.float64).sum()
    # neg pairs all have cos_sim < 1 => relu(cos_sim - margin) == 0 exactly
    loss = np.float32(pos_sum / pos_cnt + 0.0 / neg_cnt)
    return np.asarray(loss, np.float32)


_prog_cache = {}


def kernel(inputs, targets):
    from concourse.bass_utils import run_bass_kernel_spmd
    in_maps, pos_cnt, neg_cnt = host_prepare(inputs, targets)
    if "nc" not in _prog_cache:
        _prog_cache["nc"] = build_program()
    nc = _prog_cache["nc"]
    res = run_bass_kernel_spmd(nc, in_maps, list(range(NCORES)))
    stats_list = [res.results[c]["stats"] for c in range(NCORES)]
    return combine(stats_list, pos_cnt, neg_cnt)
